# revision 4
# baseline (speedup 1.0000x reference)
"""Contrastive cosine-similarity MSE loss kernel for Trainium2 (8 cores).

Math (reference): scores_n = <a_n, b_n> / (||a_n|| * ||b_n||);
loss = mean((scores - labels)^2) over N=8192 rows, D=1024.

Embeddings are downcast to fp16 on the host (cosine similarity is
scale-invariant to first order; measured end-to-end loss error vs the
fp32 reference is ~4e-7). All reductions accumulate in fp32.

Per core (1024 rows): 24 reduction passes of [128 x 1024] are needed
(dot, ||a||^2, ||b||^2 per 128-row block-half). Measured op costs on
HW: any accumulator-bearing op runs in 1x DVE/Act mode, so a pass is
~1.21us on VectorE (STT + accum read) and ~1.41us on ScalarE
(activation Square + accum read). The schedule therefore:
  - issues all 8 data-tile DMAs up front (io pool holds everything,
    no buffer reuse -> no WAR waits); the first b/a tiles are split
    into half-tile DMAs so compute starts ~0.9us in;
  - splits passes DVE 13 / ScalarE 11 (balances 13*1.21 vs 11*1.41);
  - gives each engine a private scratch tile (reused in-order on the
    same engine only -> no cross-engine serialization);
  - orders passes so each one's input tile has already landed by the
    time the engine reaches it (b-tiles are consumed first).
Tail: per-row stats [128, 8] -> (sub, mul, sqrt, recip, mul, sub,
square+accum) -> ones-matmul partition reduce -> [1,1] DMA out.
Host sums the 8 per-core scalars and divides by N.
"""

import numpy as np

import concourse.bacc as bacc
import concourse.bass as bass
import concourse.tile as tile
from concourse import mybir
from concourse.bass_utils import run_bass_kernel_spmd
from concourse.masks import make_identity
from concourse.vector_clock import ScopedClock


class _LeanTileContext(tile.TileContext):
    """TileContext with a minimal kernel epilogue.

    The stock epilogue is drain + all-engine butterfly + semaphore
    clear + second butterfly. For this single-shot kernel we only need
    the drain (all DMA queues complete, so the output is in DRAM before
    the NEFF retires); engines may retire their streams independently."""

    def _drain_and_barrier(self, tick_clock, wait_clock):
        drain_inst = self.nc.sync.drain()
        wait_clock.add_sem_waits(
            drain_inst.ins, ScopedClock({None: tick_clock.global_clock})
        )
        popped = self.nc._tile_sem_poison_stack.pop()
        assert popped is self._sem_poison

N, D = 8192, 1024
N_CORES = 8
ROWS = N // N_CORES  # rows per core
P = 128  # SBUF partitions
RPT = 2 * P  # rows per tile (2 per partition)
NTILES = ROWS // RPT  # 4
NC_ = 2 * NTILES  # stats columns (tile t, half j -> c = 2t+j)

_cache = {}


def _build():
    nc = bacc.Bacc("TRN2", target_bir_lowering=False, debug=False)

    f32 = mybir.dt.float32
    f16 = mybir.dt.float16
    a = nc.dram_tensor("a", [ROWS, D], f16, kind="ExternalInput")
    b = nc.dram_tensor("b", [ROWS, D], f16, kind="ExternalInput")
    lab = nc.dram_tensor("lab_t", [NC_, P], f32, kind="ExternalInput")
    out = nc.dram_tensor("out", [1, 1], f32, kind="ExternalOutput")

    with _LeanTileContext(nc) as tc:
        with (
            tc.tile_pool(name="io", bufs=1) as io_pool,
            tc.tile_pool(name="scr", bufs=1) as scr_pool,
            tc.tile_pool(name="psa", bufs=1, space="PSUM") as psa_pool,
            tc.tile_pool(name="stats", bufs=1) as st_pool,
        ):
            dots = st_pool.tile([P, NC_], f32)
            na = st_pool.tile([P, NC_], f32)
            nb = st_pool.tile([P, NC_], f32)

            # All 8 data tiles live simultaneously (4 MiB of SBUF) --
            # no reuse, no WAR stalls.
            at = [
                io_pool.tile([P, 2 * D], f16, tag=f"a{t}", name=f"at{t}")
                for t in range(NTILES)
            ]
            bt = [
                io_pool.tile([P, 2 * D], f16, tag=f"b{t}", name=f"bt{t}")
                for t in range(NTILES)
            ]

            def src(tsr, t, j0, nhalf):
                # partition p <- rows (t*256 + 2p, +1), halves j0..j0+nhalf
                return bass.AP(
                    tensor=tsr,
                    offset=t * RPT * D + j0 * D,
                    ap=[[2 * D, P], [1, nhalf * D]],
                )

            # Issue every data DMA up front. b0/a0 are split into
            # half-tile transfers so the first compute pass can start
            # after ~256KB instead of ~512KB.
            nc.sync.dma_start(out=bt[0][:, 0:D], in_=src(b, 0, 0, 1))
            nc.sync.dma_start(out=bt[0][:, D : 2 * D], in_=src(b, 0, 1, 1))
            nc.sync.dma_start(out=at[0][:, 0:D], in_=src(a, 0, 0, 1))
            nc.sync.dma_start(out=at[0][:, D : 2 * D], in_=src(a, 0, 1, 1))
            for t in range(1, NTILES):
                nc.sync.dma_start(out=bt[t], in_=src(b, t, 0, 2))
                nc.sync.dma_start(out=at[t], in_=src(a, t, 0, 2))

            ones = st_pool.tile([P, 1], f32)
            nc.vector.memset(ones, 1.0)
            # Warm the activation tables (Square + Sqrt) while the
            # first DMA is in flight.
            warm = st_pool.tile([P, 1], f32)
            nc.scalar.sqrt(warm, ones)

            # Labels: one fat DMA into [NC_, P], PE-transpose to [P, NC_].
            lab_sb = st_pool.tile([NC_, P], f32)
            nc.sync.dma_start(out=lab_sb, in_=lab[:, :])
            id8 = st_pool.tile([NC_, NC_], f32)
            make_identity(nc, id8)
            labt = psa_pool.tile([P, NC_], f32)
            nc.tensor.transpose(labt, lab_sb, id8)

            # Private scratch per engine; reused in-order on the same
            # engine only.
            sdve = scr_pool.tile([P, D], f16, tag="sdve")
            sact = scr_pool.tile([P, D], f16, tag="sact")

            def half(tile_, j):
                return tile_[:, j * D : (j + 1) * D]

            def dve_dot(c):
                nc.vector.scalar_tensor_tensor(
                    out=sdve, in0=half(at[c // 2], c % 2), scalar=1.0,
                    in1=half(bt[c // 2], c % 2),
                    op0=mybir.AluOpType.mult, op1=mybir.AluOpType.mult,
                    accum_out=dots[:, c : c + 1],
                )

            def dve_nb(c):
                nc.vector.scalar_tensor_tensor(
                    out=sdve, in0=half(bt[c // 2], c % 2), scalar=1.0,
                    in1=half(bt[c // 2], c % 2),
                    op0=mybir.AluOpType.mult, op1=mybir.AluOpType.mult,
                    accum_out=nb[:, c : c + 1],
                )

            def act_sq(tile_, c, dst):
                nc.scalar.activation(
                    out=sact, in_=half(tile_[c // 2], c % 2),
                    func=mybir.ActivationFunctionType.Square,
                    accum_out=dst[:, c : c + 1],
                )

            # DVE: 13 passes (8 dots + nb c=1,3,5,6,7)
            # Act: 11 passes (8 na  + nb c=0,2,4)
            # Interleaved so pass k's tile has landed before the engine
            # reaches it (b-tiles land before a-tiles of the same index).
            dve_sched = [
                ("nb", 1), ("dot", 0), ("dot", 1),
                ("nb", 3), ("dot", 2), ("dot", 3),
                ("nb", 5), ("dot", 4), ("dot", 5),
                ("nb", 7), ("nb", 6), ("dot", 6), ("dot", 7),
            ]
            act_sched = [
                ("nb", 0), ("na", 0), ("na", 1),
                ("nb", 2), ("na", 2), ("na", 3),
                ("nb", 4), ("na", 4), ("na", 5),
                ("na", 6), ("na", 7),
            ]
            ia = id_ = 0
            for kind, c in dve_sched:
                (dve_dot if kind == "dot" else dve_nb)(c)
            for kind, c in act_sched:
                act_sq(bt if kind == "nb" else at, c, nb if kind == "nb" else na)

            # Tail on [P, NC_] stats (tiny, fp32).
            prod = st_pool.tile([P, NC_], f32)
            nc.vector.tensor_mul(prod, na, nb)
            nc.scalar.sqrt(prod, prod)
            rs = st_pool.tile([P, NC_], f32)
            nc.vector.reciprocal(rs, prod)
            score = st_pool.tile([P, NC_], f32)
            nc.vector.tensor_mul(score, dots, rs)
            diff = st_pool.tile([P, NC_], f32)
            nc.vector.tensor_sub(diff, score, labt)
            sqd = st_pool.tile([P, NC_], f32)
            partial = st_pool.tile([P, 1], f32)
            nc.vector.scalar_tensor_tensor(
                out=sqd, in0=diff, scalar=1.0, in1=diff,
                op0=mybir.AluOpType.mult, op1=mybir.AluOpType.mult,
                accum_out=partial,
            )
            # Reduce 128 partitions -> [1,1] so the output DMA is one
            # descriptor instead of 128.
            total_ps = psa_pool.tile([1, 1], f32)
            nc.tensor.matmul(total_ps, partial, ones)
            res_sb = st_pool.tile([1, 1], f32)
            nc.scalar.copy(res_sb, total_ps)
            nc.sync.dma_start(out=out[:, :], in_=res_sb)

    nc.compile()
    return nc


def _label_perm(lab_core):
    """[ROWS] -> [NC_, P] so that PE-transpose yields labt[p, c] =
    labels[256*(c//2) + 2p + (c%2)], matching the stats layout."""
    return np.ascontiguousarray(
        lab_core.reshape(NTILES, P, 2).transpose(0, 2, 1).reshape(NC_, P)
    )


def kernel(issues_1_geb, issues_2_geb, labels):
    if "nc" not in _cache:
        _cache["nc"] = _build()
    nc = _cache["nc"]

    a16 = np.ascontiguousarray(issues_1_geb, dtype=np.float16)
    b16 = np.ascontiguousarray(issues_2_geb, dtype=np.float16)
    lab = np.ascontiguousarray(labels, dtype=np.float32)

    in_maps = []
    for c in range(N_CORES):
        sl = slice(c * ROWS, (c + 1) * ROWS)
        in_maps.append(
            {
                "a": np.ascontiguousarray(a16[sl]),
                "b": np.ascontiguousarray(b16[sl]),
                "lab_t": _label_perm(lab[sl]),
            }
        )

    res = run_bass_kernel_spmd(nc, in_maps, core_ids=list(range(N_CORES)))
    total = np.float64(0.0)
    for r in res.results:
        total += np.float64(r["out"].sum(dtype=np.float64))
    return np.array(total / N, dtype=np.float32)


# revision 6
# speedup vs baseline: 1.0026x; 1.0026x over previous
"""Contrastive cosine-similarity MSE loss kernel for Trainium2 (8 cores).

Math (reference): scores_n = <a_n, b_n> / (||a_n|| * ||b_n||);
loss = mean((scores - labels)^2) over N=8192 rows, D=1024.

Embeddings are downcast to fp16 on the host (cosine similarity is
scale-invariant to first order; measured end-to-end loss error vs the
fp32 reference is ~4e-7). All reductions accumulate in fp32.

Per core (1024 rows): 24 reduction passes of [128 x 1024] are needed
(dot, ||a||^2, ||b||^2 per 128-row block-half). Measured op costs on
HW: any accumulator-bearing op runs in 1x DVE/Act mode, so a pass is
~1.21us on VectorE (STT + accum read) and ~1.41us on ScalarE
(activation Square + accum read). The schedule therefore:
  - issues all 8 data-tile DMAs up front (io pool holds everything,
    no buffer reuse -> no WAR waits); the first b/a tiles are split
    into half-tile DMAs so compute starts ~0.9us in;
  - splits passes DVE 13 / ScalarE 11 (balances 13*1.21 vs 11*1.41);
  - gives each engine a private scratch tile (reused in-order on the
    same engine only -> no cross-engine serialization);
  - orders passes so each one's input tile has already landed by the
    time the engine reaches it (b-tiles are consumed first).
Tail: per-row stats [128, 8] -> (sub, mul, sqrt, recip, mul, sub,
square+accum) -> ones-matmul partition reduce -> [1,1] DMA out.
Host sums the 8 per-core scalars and divides by N.
"""

import numpy as np

import concourse.bacc as bacc
import concourse.bass as bass
import concourse.tile as tile
from concourse import mybir
from concourse.bass_utils import run_bass_kernel_spmd
from concourse.masks import make_identity
from concourse.vector_clock import ScopedClock


class _LeanTileContext(tile.TileContext):
    """TileContext with a minimal kernel epilogue.

    The stock epilogue is drain + all-engine butterfly + semaphore
    clear + second butterfly. For this single-shot kernel we only need
    the drain (all DMA queues complete, so the output is in DRAM before
    the NEFF retires); engines may retire their streams independently."""

    def _drain_and_barrier(self, tick_clock, wait_clock):
        drain_inst = self.nc.sync.drain()
        wait_clock.add_sem_waits(
            drain_inst.ins, ScopedClock({None: tick_clock.global_clock})
        )
        popped = self.nc._tile_sem_poison_stack.pop()
        assert popped is self._sem_poison

N, D = 8192, 1024
N_CORES = 8
ROWS = N // N_CORES  # rows per core
P = 128  # SBUF partitions
RPT = 2 * P  # rows per tile (2 per partition)
NTILES = ROWS // RPT  # 4
NC_ = 2 * NTILES  # stats columns (tile t, half j -> c = 2t+j)

_cache = {}


def _build():
    nc = bacc.Bacc("TRN2", target_bir_lowering=False, debug=False)

    f32 = mybir.dt.float32
    f16 = mybir.dt.float16
    a = nc.dram_tensor("a", [ROWS, D], f16, kind="ExternalInput")
    b = nc.dram_tensor("b", [ROWS, D], f16, kind="ExternalInput")
    lab = nc.dram_tensor("lab_t", [NC_, P], f32, kind="ExternalInput")
    out = nc.dram_tensor("out", [1, 1], f32, kind="ExternalOutput")

    with _LeanTileContext(nc) as tc:
        with (
            tc.tile_pool(name="io", bufs=1) as io_pool,
            tc.tile_pool(name="scr", bufs=1) as scr_pool,
            tc.tile_pool(name="psa", bufs=1, space="PSUM") as psa_pool,
            tc.tile_pool(name="stats", bufs=1) as st_pool,
        ):
            dots = st_pool.tile([P, NC_], f32)
            na = st_pool.tile([P, NC_], f32)
            nb = st_pool.tile([P, NC_], f32)

            # All 8 data tiles live simultaneously (4 MiB of SBUF) --
            # no reuse, no WAR stalls.
            at = [
                io_pool.tile([P, 2 * D], f16, tag=f"a{t}", name=f"at{t}")
                for t in range(NTILES)
            ]
            bt = [
                io_pool.tile([P, 2 * D], f16, tag=f"b{t}", name=f"bt{t}")
                for t in range(NTILES)
            ]

            def src(tsr, t, j0, nhalf):
                # partition p <- rows (t*256 + 2p, +1), halves j0..j0+nhalf
                return bass.AP(
                    tensor=tsr,
                    offset=t * RPT * D + j0 * D,
                    ap=[[2 * D, P], [1, nhalf * D]],
                )

            # Issue every data DMA up front, round-robin across three
            # otherwise-idle trigger engines so the transfers run on
            # parallel hardware DMA queues (a single queue streams at
            # only ~170 GB/s; the core sustains ~340). b0/a0 are split
            # into half-tile transfers so compute starts earlier.
            xfers = [
                (bt[0][:, 0:D], src(b, 0, 0, 1)),
                (bt[0][:, D : 2 * D], src(b, 0, 1, 1)),
                (at[0][:, 0:D], src(a, 0, 0, 1)),
                (at[0][:, D : 2 * D], src(a, 0, 1, 1)),
            ]
            for t in range(1, NTILES):
                xfers.append((bt[t], src(b, t, 0, 2)))
                xfers.append((at[t], src(a, t, 0, 2)))
            engs = [
                nc.sync, nc.gpsimd, nc.scalar, nc.sync, nc.gpsimd,
                nc.sync, nc.gpsimd, nc.sync, nc.gpsimd, nc.sync,
            ]
            for i, (dst, s) in enumerate(xfers):
                engs[i].dma_start(out=dst, in_=s)

            ones = st_pool.tile([P, 1], f32)
            nc.vector.memset(ones, 1.0)
            # Warm the activation tables (Square + Sqrt) while the
            # first DMA is in flight.
            warm = st_pool.tile([P, 1], f32)
            nc.scalar.sqrt(warm, ones)

            # Labels: one fat DMA into [NC_, P], PE-transpose to [P, NC_].
            lab_sb = st_pool.tile([NC_, P], f32)
            nc.sync.dma_start(out=lab_sb, in_=lab[:, :])
            id8 = st_pool.tile([NC_, NC_], f32)
            make_identity(nc, id8)
            labt = psa_pool.tile([P, NC_], f32)
            nc.tensor.transpose(labt, lab_sb, id8)

            # Private scratch per engine; reused in-order on the same
            # engine only.
            sdve = scr_pool.tile([P, D], f16, tag="sdve")
            sact = scr_pool.tile([P, D], f16, tag="sact")

            def half(tile_, j):
                return tile_[:, j * D : (j + 1) * D]

            def dve_dot(c):
                nc.vector.scalar_tensor_tensor(
                    out=sdve, in0=half(at[c // 2], c % 2), scalar=1.0,
                    in1=half(bt[c // 2], c % 2),
                    op0=mybir.AluOpType.mult, op1=mybir.AluOpType.mult,
                    accum_out=dots[:, c : c + 1],
                )

            def dve_nb(c):
                nc.vector.scalar_tensor_tensor(
                    out=sdve, in0=half(bt[c // 2], c % 2), scalar=1.0,
                    in1=half(bt[c // 2], c % 2),
                    op0=mybir.AluOpType.mult, op1=mybir.AluOpType.mult,
                    accum_out=nb[:, c : c + 1],
                )

            def act_sq(tile_, c, dst):
                nc.scalar.activation(
                    out=sact, in_=half(tile_[c // 2], c % 2),
                    func=mybir.ActivationFunctionType.Square,
                    accum_out=dst[:, c : c + 1],
                )

            # DVE: 13 passes (8 dots + nb c=1,3,5,6,7)
            # Act: 11 passes (8 na  + nb c=0,2,4)
            # Interleaved so pass k's tile has landed before the engine
            # reaches it (b-tiles land before a-tiles of the same index).
            dve_sched = [
                ("nb", 1), ("dot", 0), ("dot", 1),
                ("nb", 3), ("dot", 2), ("dot", 3),
                ("nb", 5), ("dot", 4), ("dot", 5),
                ("nb", 7), ("nb", 6), ("dot", 6), ("dot", 7),
            ]
            act_sched = [
                ("nb", 0), ("na", 0), ("na", 1),
                ("nb", 2), ("na", 2), ("na", 3),
                ("nb", 4), ("na", 4), ("na", 5),
                ("na", 6), ("na", 7),
            ]
            ia = id_ = 0
            for kind, c in dve_sched:
                (dve_dot if kind == "dot" else dve_nb)(c)
            for kind, c in act_sched:
                act_sq(bt if kind == "nb" else at, c, nb if kind == "nb" else na)

            # Tail on [P, NC_] stats (tiny, fp32).
            prod = st_pool.tile([P, NC_], f32)
            nc.vector.tensor_mul(prod, na, nb)
            nc.scalar.sqrt(prod, prod)
            rs = st_pool.tile([P, NC_], f32)
            nc.vector.reciprocal(rs, prod)
            score = st_pool.tile([P, NC_], f32)
            nc.vector.tensor_mul(score, dots, rs)
            diff = st_pool.tile([P, NC_], f32)
            nc.vector.tensor_sub(diff, score, labt)
            sqd = st_pool.tile([P, NC_], f32)
            partial = st_pool.tile([P, 1], f32)
            nc.vector.scalar_tensor_tensor(
                out=sqd, in0=diff, scalar=1.0, in1=diff,
                op0=mybir.AluOpType.mult, op1=mybir.AluOpType.mult,
                accum_out=partial,
            )
            # Reduce 128 partitions -> [1,1] so the output DMA is one
            # descriptor instead of 128.
            total_ps = psa_pool.tile([1, 1], f32)
            nc.tensor.matmul(total_ps, partial, ones)
            res_sb = st_pool.tile([1, 1], f32)
            nc.scalar.copy(res_sb, total_ps)
            nc.sync.dma_start(out=out[:, :], in_=res_sb)

    nc.compile()
    return nc


def _label_perm(lab_core):
    """[ROWS] -> [NC_, P] so that PE-transpose yields labt[p, c] =
    labels[256*(c//2) + 2p + (c%2)], matching the stats layout."""
    return np.ascontiguousarray(
        lab_core.reshape(NTILES, P, 2).transpose(0, 2, 1).reshape(NC_, P)
    )


def kernel(issues_1_geb, issues_2_geb, labels):
    if "nc" not in _cache:
        _cache["nc"] = _build()
    nc = _cache["nc"]

    a16 = np.ascontiguousarray(issues_1_geb, dtype=np.float16)
    b16 = np.ascontiguousarray(issues_2_geb, dtype=np.float16)
    lab = np.ascontiguousarray(labels, dtype=np.float32)

    in_maps = []
    for c in range(N_CORES):
        sl = slice(c * ROWS, (c + 1) * ROWS)
        in_maps.append(
            {
                "a": np.ascontiguousarray(a16[sl]),
                "b": np.ascontiguousarray(b16[sl]),
                "lab_t": _label_perm(lab[sl]),
            }
        )

    res = run_bass_kernel_spmd(nc, in_maps, core_ids=list(range(N_CORES)))
    total = np.float64(0.0)
    for r in res.results:
        total += np.float64(r["out"].sum(dtype=np.float64))
    return np.array(total / N, dtype=np.float32)
